# revision 19
# baseline (speedup 1.0000x reference)
"""Trainium2 Bass kernel for BaselineFeedforwardNetwork recurrence.

Reference computation (per path, T=60 steps, serial in t):
    x_t = [features_t (8), delta_{t-1} (1)]            # (9,)
    h1  = relu(x_t @ W1 + b1)                          # (128,)
    h2  = relu(h1 @ W2 + b2)                           # (128,)
    d_t = h2 @ W3 + b3                                 # (1,)
Output: deltas (N, T).

Data-parallel over N=65536 paths across 8 NeuronCores (8192/core),
weights replicated, recurrence local per core.

v8: col-tiled mm3 wave + per-pack pd + DEEP pipeline: slot P runs
[mm3(P-3), mm2(P-2), mm1(P)] so every PSUM eviction has a full slot to
drain before its consumer issues -- PE and the eviction engines overlap
instead of ping-ponging.

The kernel is bound by the PSUM->SBUF eviction engines (ACT+DVE are the
only engines that can read PSUM; each moves ~1 fp32 column/cycle).  v5
minimizes both PE work and eviction op count:
- mm1: K=9 fused (delta row scattered into the feature tile), 4x
  row-tiled concurrent wave into 2 psum PAIR tiles; h1 eviction is one
  1024-wide op per pair.  (unchanged from v4)
- mm2: full K=128 per 512-tile; h2 eviction per tile.
- mm3: 4x COL-TILED concurrent wave: for group g, lhsT is a 10-column
  slice of W3st ([128,11], col 9 = W3) at tile_position (0, 32g), so
  the four group matmuls run concurrently (~216ns vs ~1280ns serial).
  Packs are PAIRED into one pd psum bank: pack parity s'=0 uses slice
  w3st[:,1:11] (delta -> psum row 32g+8), s'=1 uses w3st[:,0:10]
  (delta -> row 32g+9) and accumulates (start=False) so the pair
  shares one [128,512] bank.  ONE pd eviction per 2 packs (b3 added
  here), then 2 scatter DMAs (per sub-parity) into the next step's
  delta rows and 2 out DMAs.
- PSUM: ph1 2x[128,1024] (4 banks) + ph2 3x[128,512] + pd 1x[128,512]
  = 8 banks.
- reps>1 repeats the whole T-step recurrence for timing (slope method).
"""

import os
import sys

import numpy as np

for _p in ("/opt/trn_rl_repo", "/root/.axon_site/_ro/trn_rl_repo"):
    if _p not in sys.path and os.path.isdir(_p):
        sys.path.append(_p)

import ml_dtypes  # noqa: E402

N_FULL = 65536
T_FULL = 60
F = 8
HID = 128
N_CORES = 8
NSH = N_FULL // N_CORES  # 8192 paths per core

BF16 = ml_dtypes.bfloat16


def build_kernel(nsh=NSH, t_steps=T_FULL, num_cores=N_CORES, b3_value=0.0,
                 chunk_steps=15, reps=1, tiny_evict=False, tiny_pe=False,
                 pd_start_each=True):
    """Builds the per-core Bass graph. Returns the compiled nc."""
    import concourse.bass as bass
    import concourse.tile as tile
    from concourse import bacc, mybir

    bf = mybir.dt.bfloat16
    f32 = mybir.dt.float32
    NT = 512                       # path-tile width (one fp32 psum bank)
    ntiles = nsh // NT
    npacks = ntiles // 4           # pack = 4 row-tiled tiles
    xw = npacks * NT               # per-step free width
    TC = min(chunk_steps, t_steps)
    nchunks = (t_steps + TC - 1) // TC
    assert ntiles % 4 == 0
    assert npacks % 2 == 0

    nc = bacc.Bacc(
        "TRN2", target_bir_lowering=False, debug=False,
        num_devices=num_cores,
    )

    feat = nc.declare_dram_parameter("features", [4, F, t_steps, xw], bf, isOutput=False)
    w1c = nc.declare_dram_parameter("W1c", [128, HID], bf, isOutput=False)
    w2 = nc.declare_dram_parameter("W2", [HID, HID], bf, isOutput=False)
    w3st_d = nc.declare_dram_parameter("W3st", [128, 4, 1], bf, isOutput=False)
    b1 = nc.declare_dram_parameter("b1", [128, 1], f32, isOutput=False)
    b2 = nc.declare_dram_parameter("b2", [128, 1], f32, isOutput=False)
    # out[t, s, g, c] = delta of path tile (s, g), col c
    out = nc.declare_dram_parameter(
        "out", [t_steps, npacks, 4, NT], bf, isOutput=True)

    Relu = mybir.ActivationFunctionType.Relu
    Copy = mybir.ActivationFunctionType.Copy
    add = mybir.AluOpType.add
    amax = mybir.AluOpType.max

    NP_TOT = reps * t_steps * npacks   # total pack count in the pipeline

    with tile.TileContext(nc) as tc:
        with (
            tc.tile_pool(name="consts", bufs=1) as cpool,
            tc.tile_pool(name="f", bufs=2) as fpool,
            tc.tile_pool(name="h1r", bufs=3) as h1pool,
            tc.tile_pool(name="h2r", bufs=8) as h2pool,
            tc.tile_pool(name="dst", bufs=2) as dpool,
            tc.tile_pool(name="ph1", bufs=1, space="PSUM") as ph1pool,
            tc.tile_pool(name="ph2", bufs=3, space="PSUM") as ph2pool,
            tc.tile_pool(name="pd", bufs=1, space="PSUM") as pdpool,
        ):
            w1sb = cpool.tile([128, HID], bf, tag="w1")
            w2sb = cpool.tile([HID, HID], bf, tag="w2")
            w3sb = cpool.tile([128, 4, 1], bf, tag="w3")
            b1sb = cpool.tile([128, 1], f32, tag="b1")
            b2sb = cpool.tile([128, 1], f32, tag="b2")
            nc.gpsimd.dma_start(w1sb[:], w1c[:])
            nc.gpsimd.dma_start(w2sb[:], w2[:])
            nc.gpsimd.dma_start(w3sb[:], w3st_d[:])
            nc.gpsimd.dma_start(b1sb[:], b1[:])
            nc.gpsimd.dma_start(b2sb[:], b2[:])

            ftile_by_gc = {}

            def load_chunk(gc):
                ci = gc % nchunks
                t0 = ci * TC
                tl = min(TC, t_steps - t0)
                ftile = fpool.tile([128, TC * xw], bf, tag="f")
                for g in range(4):
                    nc.sync.dma_start(
                        ftile[32 * g:32 * g + 8, 0:tl * xw],
                        feat[g, 0:F, t0:t0 + tl, 0:xw],
                    )
                ftile_by_gc[gc] = ftile
                if gc - 2 in ftile_by_gc:
                    del ftile_by_gc[gc - 2]
                return ftile

            fcur = None
            fnxt = None
            h1aps_by_p = {}       # global pack idx -> list of 4 h1 slices
            h2r_by_p = {}         # global pack idx -> list of 4 h2r tiles

            def stage_mm1(P, fcur):
                """K=9 fused mm1 wave + h1 pair evictions for pack P."""
                gt, s = divmod(P, npacks)
                first = gt == 0
                tt = (gt % t_steps) % TC
                kk = 8 if first else 9   # no delta row at t=0
                ph1 = ph1pool.tile([128, 4 * NT], f32, tag="ph1",
                                   name=f"ph1p{P}")
                PW = 8 if tiny_pe else NT
                for g in range(4):
                    fs = tt * xw + NT * s
                    nc.tensor.matmul(
                        ph1[:, NT * g:NT * g + PW],
                        lhsT=w1sb[32 * g:32 * g + kk, :],
                        rhs=fcur[32 * g:32 * g + kk, fs:fs + PW],
                        start=True, stop=True,
                        tile_position=(32 * g, 0),
                    )
                EW = 4 * NT if not tiny_evict else 8
                h1r = h1pool.tile([128, 4 * NT], bf, tag="h1r",
                                  name=f"h1r{P}")
                if P % 2 == 0:
                    nc.scalar.activation(h1r[:, 0:EW], ph1[:, 0:EW],
                                         Relu, bias=b1sb[:, 0:1])
                else:
                    nc.vector.tensor_scalar(h1r[:, 0:EW], ph1[:, 0:EW],
                                            b1sb[:, 0:1], 0.0, add, amax)
                h1aps_by_p[P] = [(h1r, NT * g) for g in range(4)]

            def stage_mm2(P):
                """mm2 + h2 evictions for global pack P."""
                h1aps = h1aps_by_p.pop(P)
                h2rs = []
                PW = 8 if tiny_pe else NT
                EW = 8 if tiny_evict else NT
                for g in range(4):
                    h1t, off = h1aps[g]
                    ph2 = ph2pool.tile([128, NT], f32, tag="ph2")
                    nc.tensor.matmul(
                        ph2[:, 0:PW], lhsT=w2sb[:],
                        rhs=h1t[:, off:off + PW],
                        start=True, stop=True)
                    h2r = h2pool.tile([128, NT], bf, tag="h2r")
                    if (g + P) % 2 == 0:
                        nc.vector.tensor_scalar(h2r[:, 0:EW], ph2[:, 0:EW],
                                                b2sb[:, 0:1], 0.0, add, amax)
                    else:
                        nc.scalar.activation(h2r[:, 0:EW], ph2[:, 0:EW], Relu,
                                             bias=b2sb[:, 0:1])
                    h2rs.append(h2r)
                h2r_by_p[P] = h2rs

            def stage_mm3(P):
                """Col-tiled mm3 wave + per-pack pd evict, scatter, out."""
                gt, s = divmod(P, npacks)
                rep, t = divmod(gt, t_steps)
                h2rs = h2r_by_p.pop(P)
                pd = pdpool.tile([128, NT], f32, tag="pd", name=f"pd{P}")
                PW = 8 if tiny_pe else NT
                for g in range(4):
                    nc.tensor.matmul(
                        pd[32 * g:32 * g + 1, 0:PW],
                        lhsT=w3sb[:, g, 0:1],
                        rhs=h2rs[g][:, 0:PW],
                        start=True, stop=True,
                        tile_position=(0, 32 * g),
                        skip_group_check=True,
                    )
                # pd eviction (adds b3), then scatter + out DMA
                ds = dpool.tile([128, NT], bf, tag="dst", name=f"ds{P}")
                EW = 8 if tiny_evict else NT
                if P % 2 == 0:
                    nc.scalar.activation(ds[:, 0:EW], pd[:, 0:EW], Copy,
                                         bias=float(b3_value))
                else:
                    nc.vector.tensor_scalar(ds[:, 0:EW], pd[:, 0:EW],
                                            float(b3_value), None, add)
                if gt + 1 < reps * t_steps:
                    rep1, t1 = divmod(gt + 1, t_steps)
                    tt1 = t1 % TC
                    gc1 = rep1 * nchunks + t1 // TC
                    ft1 = ftile_by_gc[gc1]
                    fs1 = tt1 * xw + NT * s
                    eng = nc.sync if s % 2 == 0 else nc.gpsimd
                    eng.dma_start(
                        ft1[8:105:32, fs1:fs1 + NT],
                        ds[0:97:32, 0:NT],
                    )
                eng2 = nc.gpsimd if s % 2 == 0 else nc.sync
                eng2.dma_start(out[t, s, 0:4, 0:NT], ds[0:97:32, 0:NT])

            for P in range(NP_TOT + 3):
                # oldest work first: mm3(P-3), mm2(P-2), then the new wave.
                # Each eviction gets a full slot before its consumer issues.
                if 0 <= P - 3 < NP_TOT:
                    stage_mm3(P - 3)
                if 0 <= P - 2 < NP_TOT:
                    stage_mm2(P - 2)
                if P < NP_TOT:
                    gt, s = divmod(P, npacks)
                    rep, t = divmod(gt, t_steps)
                    tt = t % TC
                    ci = t // TC
                    if s == 0:
                        if P == 0:
                            fcur = load_chunk(0)
                        elif tt == 0:
                            fcur = fnxt
                        if tt == 0:
                            gc = rep * nchunks + ci
                            if (ci + 1) * TC < t_steps or rep + 1 < reps:
                                fnxt = load_chunk(gc + 1)
                    stage_mm1(P, fcur)

    nc.compile()
    return nc


_NC_CACHE = {}


def _get_nc(nsh=NSH, t_steps=T_FULL, num_cores=N_CORES, b3_value=0.0, **kw):
    key = (nsh, t_steps, num_cores, float(b3_value), tuple(sorted(kw.items())))
    if key not in _NC_CACHE:
        _NC_CACHE[key] = build_kernel(nsh, t_steps, num_cores, b3_value, **kw)
    return _NC_CACHE[key]


def prep_core_inputs(features, W1, b1, W2, b2, W3, b3, num_cores=N_CORES):
    """Host-side shard + repack. Returns list of per-core in_maps."""
    n, t_steps, f = features.shape
    nsh = n // num_cores
    NT = 512
    npacks = nsh // (4 * NT)
    xw = npacks * NT

    w1c = np.zeros((128, HID), dtype=BF16)
    for g in range(4):
        w1c[32 * g:32 * g + 8, :] = W1[0:8].astype(BF16)
        w1c[32 * g + 8, :] = W1[8].astype(BF16)
    w3st = W3[:, 0:1].astype(BF16).reshape(128, 1, 1).repeat(4, axis=1)
    w2b = W2.astype(BF16)
    b1c = b1.reshape(128, 1).astype(np.float32)
    b2c = b2.reshape(128, 1).astype(np.float32)

    in_maps = []
    for c in range(num_cores):
        fc = features[c * nsh:(c + 1) * nsh]          # (nsh, T, F)
        # path p = 2048s + 512g + c_ ; fpk[g, k, t, 512s + c_]
        fpk = fc.reshape(npacks, 4, NT, t_steps, f)   # (s, g, c_, t, k)
        fpk = fpk.transpose(1, 4, 3, 0, 2).reshape(4, f, t_steps, xw)
        in_maps.append({
            "features": np.ascontiguousarray(fpk).astype(BF16),
            "W1c": w1c, "W2": w2b, "W3st": w3st,
            "b1": b1c, "b2": b2c,
        })
    return in_maps


def gather_out(res_core, nsh, t_steps):
    """(T, npacks, 4, 512) bf16 -> (nsh, T) fp32, path p = 2048s+512g+c."""
    o = np.asarray(res_core).astype(np.float32)       # (t, s, g, c)
    o = o.transpose(1, 2, 3, 0)                       # (s, g, c, t)
    return o.reshape(nsh, t_steps)


def run(features, W1, b1, W2, b2, W3, b3, **run_kwargs):
    """Run on the 8 cores; returns (full_output, BassKernelResults)."""
    from concourse.bass_utils import run_bass_kernel_spmd

    features = np.asarray(features)
    n, t_steps, f = features.shape
    nsh = n // N_CORES
    b3v = float(np.asarray(b3).reshape(-1)[0])
    in_maps = prep_core_inputs(features, W1, b1, W2, b2, W3, b3)
    nc = _get_nc(nsh, t_steps, N_CORES, b3v)
    res = run_bass_kernel_spmd(nc, in_maps, core_ids=list(range(N_CORES)), **run_kwargs)
    outs = [gather_out(res.results[c]["out"], nsh, t_steps)
            for c in range(N_CORES)]
    return np.concatenate(outs, axis=0), res


def kernel(features, W1, b1, W2, b2, W3, b3):
    out, _ = run(features, W1, b1, W2, b2, W3, b3)
    return out


# revision 20
# speedup vs baseline: 1.1645x; 1.1645x over previous
"""Trainium2 Bass kernel for BaselineFeedforwardNetwork recurrence.

Reference computation (per path, T=60 steps, serial in t):
    x_t = [features_t (8), delta_{t-1} (1)]            # (9,)
    h1  = relu(x_t @ W1 + b1)                          # (128,)
    h2  = relu(h1 @ W2 + b2)                           # (128,)
    d_t = h2 @ W3 + b3                                 # (1,)
Output: deltas (N, T).

Data-parallel over N=65536 paths across 8 NeuronCores (8192/core),
weights replicated, recurrence local per core.

v8: col-tiled mm3 wave + per-pack pd + DEEP pipeline: slot P runs
[mm3(P-3), mm2(P-2), mm1(P)] so every PSUM eviction has a full slot to
drain before its consumer issues -- PE and the eviction engines
overlap instead of ping-ponging.  (v6/v11 DMA-free delta feedback and
v10/v12 granularity changes all measured slower; see session notes.)

The kernel is limited by the PSUM->SBUF eviction engines (ACT+DVE are
the only engines that can read PSUM; ~1 fp32 column/cycle each) plus
the per-step serial delta chain.  Design:
- mm1: K=9 fused (delta row scattered into the feature tile), 4x
  row-tiled concurrent wave (tile_position (32g,0)) into 2 psum PAIR
  tiles; h1 eviction is one 1024-wide op per pair, one on ACT and one
  on DVE so the pair drains in parallel (~1.2us wall).
- mm2: full K=128 per 512-tile into a 3-buf psum pool; h2 eviction per
  tile, engines alternating.
- mm3: 4x COL-TILED concurrent wave: for group g, lhsT is the single
  W3 column at tile_position (0, 32g), so the delta lands on psum
  partition 32g and the four group matmuls share one [128,512] bank
  and run concurrently (~0.25us vs ~1.3us serial block-diagonal).
  Per-pack pd eviction (adds b3) -> ds, then 1 scatter DMA into the
  next step's delta row (32g+8) of the feature tile + 1 out DMA.
- DEEP pipeline: slot P emits [mm3(P-3), mm2(P-2), mm1(P)] so every
  psum eviction gets a full slot to drain before its consumer issues
  and the delta scatter has a slot of slack -- PE and the eviction
  engines overlap instead of ping-ponging (v4/v5 ran them lockstep).
- PSUM: ph1 2x[128,1024] (4 banks) + ph2 3x[128,512] + pd 1x[128,512]
  = 8 banks.
- reps>1 repeats the whole T-step recurrence for timing (slope method).
"""

import os
import sys

import numpy as np

for _p in ("/opt/trn_rl_repo", "/root/.axon_site/_ro/trn_rl_repo"):
    if _p not in sys.path and os.path.isdir(_p):
        sys.path.append(_p)

import ml_dtypes  # noqa: E402

N_FULL = 65536
T_FULL = 60
F = 8
HID = 128
N_CORES = 8
NSH = N_FULL // N_CORES  # 8192 paths per core

BF16 = ml_dtypes.bfloat16


def build_kernel(nsh=NSH, t_steps=T_FULL, num_cores=N_CORES, b3_value=0.0,
                 chunk_steps=15, reps=1, tiny_evict=False, tiny_pe=False,
                 pd_start_each=True):
    """Builds the per-core Bass graph. Returns the compiled nc."""
    import concourse.bass as bass
    import concourse.tile as tile
    from concourse import bacc, mybir

    bf = mybir.dt.bfloat16
    f32 = mybir.dt.float32
    NT = 512                       # path-tile width (one fp32 psum bank)
    ntiles = nsh // NT
    npacks = ntiles // 4           # pack = 4 row-tiled tiles
    xw = npacks * NT               # per-step free width
    TC = min(chunk_steps, t_steps)
    nchunks = (t_steps + TC - 1) // TC
    assert ntiles % 4 == 0
    assert npacks % 2 == 0

    nc = bacc.Bacc(
        "TRN2", target_bir_lowering=False, debug=False,
        num_devices=num_cores,
    )

    feat = nc.declare_dram_parameter("features", [4, F, t_steps, xw], bf, isOutput=False)
    w1c = nc.declare_dram_parameter("W1c", [128, HID], bf, isOutput=False)
    w2 = nc.declare_dram_parameter("W2", [HID, HID], bf, isOutput=False)
    w3st_d = nc.declare_dram_parameter("W3st", [128, 4, 1], bf, isOutput=False)
    b1 = nc.declare_dram_parameter("b1", [128, 1], f32, isOutput=False)
    b2 = nc.declare_dram_parameter("b2", [128, 1], f32, isOutput=False)
    # out[t, s, g, c] = delta of path tile (s, g), col c
    out = nc.declare_dram_parameter(
        "out", [t_steps, npacks, 4, NT], bf, isOutput=True)

    Relu = mybir.ActivationFunctionType.Relu
    Copy = mybir.ActivationFunctionType.Copy
    add = mybir.AluOpType.add
    amax = mybir.AluOpType.max

    NP_TOT = reps * t_steps * npacks   # total pack count in the pipeline

    with tile.TileContext(nc) as tc:
        with (
            tc.tile_pool(name="consts", bufs=1) as cpool,
            tc.tile_pool(name="f", bufs=2) as fpool,
            tc.tile_pool(name="h1r", bufs=6) as h1pool,
            tc.tile_pool(name="h2r", bufs=8) as h2pool,
            tc.tile_pool(name="dst", bufs=2) as dpool,
            tc.tile_pool(name="ph1", bufs=2, space="PSUM") as ph1pool,
            tc.tile_pool(name="ph2", bufs=3, space="PSUM") as ph2pool,
            tc.tile_pool(name="pd", bufs=1, space="PSUM") as pdpool,
        ):
            w1sb = cpool.tile([128, HID], bf, tag="w1")
            w2sb = cpool.tile([HID, HID], bf, tag="w2")
            w3sb = cpool.tile([128, 4, 1], bf, tag="w3")
            b1sb = cpool.tile([128, 1], f32, tag="b1")
            b2sb = cpool.tile([128, 1], f32, tag="b2")
            nc.gpsimd.dma_start(w1sb[:], w1c[:])
            nc.gpsimd.dma_start(w2sb[:], w2[:])
            nc.gpsimd.dma_start(w3sb[:], w3st_d[:])
            nc.gpsimd.dma_start(b1sb[:], b1[:])
            nc.gpsimd.dma_start(b2sb[:], b2[:])

            ftile_by_gc = {}

            def load_chunk(gc):
                ci = gc % nchunks
                t0 = ci * TC
                tl = min(TC, t_steps - t0)
                ftile = fpool.tile([128, TC * xw], bf, tag="f")
                for g in range(4):
                    nc.sync.dma_start(
                        ftile[32 * g:32 * g + 8, 0:tl * xw],
                        feat[g, 0:F, t0:t0 + tl, 0:xw],
                    )
                ftile_by_gc[gc] = ftile
                if gc - 2 in ftile_by_gc:
                    del ftile_by_gc[gc - 2]
                return ftile

            fcur = None
            fnxt = None
            h1aps_by_p = {}       # global pack idx -> list of 4 h1 slices
            h2r_by_p = {}         # global pack idx -> list of 4 h2r tiles

            def stage_mm1(P, fcur):
                """K=9 fused mm1 wave + h1 pair evictions for pack P."""
                gt, s = divmod(P, npacks)
                first = gt == 0
                tt = (gt % t_steps) % TC
                kk = 8 if first else 9   # no delta row at t=0
                pairs = [ph1pool.tile([128, 2 * NT], f32, tag="ph1",
                                      name=f"ph1p{P}_{p}")
                         for p in range(2)]
                PW = 8 if tiny_pe else NT
                for g in range(4):
                    fs = tt * xw + NT * s
                    nc.tensor.matmul(
                        pairs[g // 2][:, NT * (g % 2):NT * (g % 2) + PW],
                        lhsT=w1sb[32 * g:32 * g + kk, :],
                        rhs=fcur[32 * g:32 * g + kk, fs:fs + PW],
                        start=True, stop=True,
                        tile_position=(32 * g, 0),
                    )
                h1aps = []
                EW = 2 * NT if not tiny_evict else 8
                for p in range(2):
                    h1r = h1pool.tile([128, 2 * NT], bf, tag="h1r",
                                      name=f"h1r{P}_{p}")
                    if (p + P) % 2 == 0:
                        nc.scalar.activation(h1r[:, 0:EW], pairs[p][:, 0:EW],
                                             Relu, bias=b1sb[:, 0:1])
                    else:
                        nc.vector.tensor_scalar(h1r[:, 0:EW], pairs[p][:, 0:EW],
                                                b1sb[:, 0:1], 0.0, add, amax)
                    h1aps += [(h1r, 0), (h1r, NT)]
                h1aps_by_p[P] = h1aps

            def stage_mm2(P):
                """mm2 + h2 evictions for global pack P."""
                h1aps = h1aps_by_p.pop(P)
                h2rs = []
                PW = 8 if tiny_pe else NT
                EW = 8 if tiny_evict else NT
                for g in range(4):
                    h1t, off = h1aps[g]
                    ph2 = ph2pool.tile([128, NT], f32, tag="ph2")
                    nc.tensor.matmul(
                        ph2[:, 0:PW], lhsT=w2sb[:],
                        rhs=h1t[:, off:off + PW],
                        start=True, stop=True)
                    h2r = h2pool.tile([128, NT], bf, tag="h2r")
                    if (g + P) % 2 == 0:
                        nc.vector.tensor_scalar(h2r[:, 0:EW], ph2[:, 0:EW],
                                                b2sb[:, 0:1], 0.0, add, amax)
                    else:
                        nc.scalar.activation(h2r[:, 0:EW], ph2[:, 0:EW], Relu,
                                             bias=b2sb[:, 0:1])
                    h2rs.append(h2r)
                h2r_by_p[P] = h2rs

            def stage_mm3(P):
                """Col-tiled mm3 wave + per-pack pd evict, scatter, out."""
                gt, s = divmod(P, npacks)
                rep, t = divmod(gt, t_steps)
                h2rs = h2r_by_p.pop(P)
                pd = pdpool.tile([128, NT], f32, tag="pd", name=f"pd{P}")
                PW = 8 if tiny_pe else NT
                for g in range(4):
                    nc.tensor.matmul(
                        pd[32 * g:32 * g + 1, 0:PW],
                        lhsT=w3sb[:, g, 0:1],
                        rhs=h2rs[g][:, 0:PW],
                        start=True, stop=True,
                        tile_position=(0, 32 * g),
                        skip_group_check=True,
                    )
                # pd eviction (adds b3), then scatter + out DMA
                ds = dpool.tile([128, NT], bf, tag="dst", name=f"ds{P}")
                EW = 8 if tiny_evict else NT
                if P % 2 == 0:
                    nc.scalar.activation(ds[:, 0:EW], pd[:, 0:EW], Copy,
                                         bias=float(b3_value))
                else:
                    nc.vector.tensor_scalar(ds[:, 0:EW], pd[:, 0:EW],
                                            float(b3_value), None, add)
                if gt + 1 < reps * t_steps:
                    rep1, t1 = divmod(gt + 1, t_steps)
                    tt1 = t1 % TC
                    gc1 = rep1 * nchunks + t1 // TC
                    ft1 = ftile_by_gc[gc1]
                    fs1 = tt1 * xw + NT * s
                    eng = nc.sync if s % 2 == 0 else nc.gpsimd
                    eng.dma_start(
                        ft1[8:105:32, fs1:fs1 + NT],
                        ds[0:97:32, 0:NT],
                    )
                eng2 = nc.gpsimd if s % 2 == 0 else nc.sync
                eng2.dma_start(out[t, s, 0:4, 0:NT], ds[0:97:32, 0:NT])

            for P in range(NP_TOT + 3):
                # oldest work first: mm3(P-3), mm2(P-2), then the new wave.
                # Each eviction gets a full slot before its consumer issues.
                if 0 <= P - 3 < NP_TOT:
                    stage_mm3(P - 3)
                if 0 <= P - 2 < NP_TOT:
                    stage_mm2(P - 2)
                if P < NP_TOT:
                    gt, s = divmod(P, npacks)
                    rep, t = divmod(gt, t_steps)
                    tt = t % TC
                    ci = t // TC
                    if s == 0:
                        if P == 0:
                            fcur = load_chunk(0)
                        elif tt == 0:
                            fcur = fnxt
                        if tt == 0:
                            gc = rep * nchunks + ci
                            if (ci + 1) * TC < t_steps or rep + 1 < reps:
                                fnxt = load_chunk(gc + 1)
                    stage_mm1(P, fcur)

    nc.compile()
    return nc


_NC_CACHE = {}


def _get_nc(nsh=NSH, t_steps=T_FULL, num_cores=N_CORES, b3_value=0.0, **kw):
    key = (nsh, t_steps, num_cores, float(b3_value), tuple(sorted(kw.items())))
    if key not in _NC_CACHE:
        _NC_CACHE[key] = build_kernel(nsh, t_steps, num_cores, b3_value, **kw)
    return _NC_CACHE[key]


def prep_core_inputs(features, W1, b1, W2, b2, W3, b3, num_cores=N_CORES):
    """Host-side shard + repack. Returns list of per-core in_maps."""
    n, t_steps, f = features.shape
    nsh = n // num_cores
    NT = 512
    npacks = nsh // (4 * NT)
    xw = npacks * NT

    w1c = np.zeros((128, HID), dtype=BF16)
    for g in range(4):
        w1c[32 * g:32 * g + 8, :] = W1[0:8].astype(BF16)
        w1c[32 * g + 8, :] = W1[8].astype(BF16)
    w3st = W3[:, 0:1].astype(BF16).reshape(128, 1, 1).repeat(4, axis=1)
    w2b = W2.astype(BF16)
    b1c = b1.reshape(128, 1).astype(np.float32)
    b2c = b2.reshape(128, 1).astype(np.float32)

    in_maps = []
    for c in range(num_cores):
        fc = features[c * nsh:(c + 1) * nsh]          # (nsh, T, F)
        # path p = 2048s + 512g + c_ ; fpk[g, k, t, 512s + c_]
        fpk = fc.reshape(npacks, 4, NT, t_steps, f)   # (s, g, c_, t, k)
        fpk = fpk.transpose(1, 4, 3, 0, 2).reshape(4, f, t_steps, xw)
        in_maps.append({
            "features": np.ascontiguousarray(fpk).astype(BF16),
            "W1c": w1c, "W2": w2b, "W3st": w3st,
            "b1": b1c, "b2": b2c,
        })
    return in_maps


def gather_out(res_core, nsh, t_steps):
    """(T, npacks, 4, 512) bf16 -> (nsh, T) fp32, path p = 2048s+512g+c."""
    o = np.asarray(res_core).astype(np.float32)       # (t, s, g, c)
    o = o.transpose(1, 2, 3, 0)                       # (s, g, c, t)
    return o.reshape(nsh, t_steps)


def run(features, W1, b1, W2, b2, W3, b3, **run_kwargs):
    """Run on the 8 cores; returns (full_output, BassKernelResults)."""
    from concourse.bass_utils import run_bass_kernel_spmd

    features = np.asarray(features)
    n, t_steps, f = features.shape
    nsh = n // N_CORES
    b3v = float(np.asarray(b3).reshape(-1)[0])
    in_maps = prep_core_inputs(features, W1, b1, W2, b2, W3, b3)
    nc = _get_nc(nsh, t_steps, N_CORES, b3v)
    res = run_bass_kernel_spmd(nc, in_maps, core_ids=list(range(N_CORES)), **run_kwargs)
    outs = [gather_out(res.results[c]["out"], nsh, t_steps)
            for c in range(N_CORES)]
    return np.concatenate(outs, axis=0), res


def kernel(features, W1, b1, W2, b2, W3, b3):
    out, _ = run(features, W1, b1, W2, b2, W3, b3)
    return out


# revision 21
# speedup vs baseline: 1.9822x; 1.7022x over previous
"""Trainium2 Bass kernel for BaselineFeedforwardNetwork recurrence.

Reference computation (per path, T=60 steps, serial in t):
    x_t = [features_t (8), delta_{t-1} (1)]            # (9,)
    h1  = relu(x_t @ W1 + b1)                          # (128,)
    h2  = relu(h1 @ W2 + b2)                           # (128,)
    d_t = h2 @ W3 + b3                                 # (1,)
Output: deltas (N, T).

Data-parallel over N=65536 paths across 8 NeuronCores (8192/core),
weights replicated, recurrence local per core.

v13: v8's deep pipeline [mm3(P-3), mm2(P-2), mm1(P)] + pd/ph2 bank
time-sharing: pd is written one slot after the pack's mm2 banks are
evicted, so it allocates from the SAME [128,1024] ph2 buffer ring
instead of its own bank.  The freed bank widens ph2 to 2x[128,1024],
halving the h2 eviction op count (8x1024 instead of 16x512 per step)
on the bottleneck ACT/DVE engines.

The kernel is bound by the PSUM->SBUF eviction engines (ACT+DVE are the
only engines that can read PSUM; each moves ~1 fp32 column/cycle).  v5
minimizes both PE work and eviction op count:
- mm1: K=9 fused (delta row scattered into the feature tile), 4x
  row-tiled concurrent wave into 2 psum PAIR tiles; h1 eviction is one
  1024-wide op per pair.  (unchanged from v4)
- mm2: full K=128 per 512-tile; h2 eviction per tile.
- mm3: 4x COL-TILED concurrent wave: for group g, lhsT is a 10-column
  slice of W3st ([128,11], col 9 = W3) at tile_position (0, 32g), so
  the four group matmuls run concurrently (~216ns vs ~1280ns serial).
  Packs are PAIRED into one pd psum bank: pack parity s'=0 uses slice
  w3st[:,1:11] (delta -> psum row 32g+8), s'=1 uses w3st[:,0:10]
  (delta -> row 32g+9) and accumulates (start=False) so the pair
  shares one [128,512] bank.  ONE pd eviction per 2 packs (b3 added
  here), then 2 scatter DMAs (per sub-parity) into the next step's
  delta rows and 2 out DMAs.
- PSUM: ph1 2x[128,1024] (4 banks) + ph2 3x[128,512] + pd 1x[128,512]
  = 8 banks.
- reps>1 repeats the whole T-step recurrence for timing (slope method).
"""

import os
import sys

import numpy as np

for _p in ("/opt/trn_rl_repo", "/root/.axon_site/_ro/trn_rl_repo"):
    if _p not in sys.path and os.path.isdir(_p):
        sys.path.append(_p)

import ml_dtypes  # noqa: E402

N_FULL = 65536
T_FULL = 60
F = 8
HID = 128
N_CORES = 8
NSH = N_FULL // N_CORES  # 8192 paths per core

BF16 = ml_dtypes.bfloat16


def build_kernel(nsh=NSH, t_steps=T_FULL, num_cores=N_CORES, b3_value=0.0,
                 chunk_steps=15, reps=1, tiny_evict=False, tiny_pe=False,
                 pd_start_each=True):
    """Builds the per-core Bass graph. Returns the compiled nc."""
    import concourse.bass as bass
    import concourse.tile as tile
    from concourse import bacc, mybir

    bf = mybir.dt.bfloat16
    f32 = mybir.dt.float32
    NT = 512                       # path-tile width (one fp32 psum bank)
    ntiles = nsh // NT
    npacks = ntiles // 4           # pack = 4 row-tiled tiles
    xw = npacks * NT               # per-step free width
    TC = min(chunk_steps, t_steps)
    nchunks = (t_steps + TC - 1) // TC
    assert ntiles % 4 == 0
    assert npacks % 2 == 0

    nc = bacc.Bacc(
        "TRN2", target_bir_lowering=False, debug=False,
        num_devices=num_cores,
    )

    feat = nc.declare_dram_parameter("features", [4, F, t_steps, xw], bf, isOutput=False)
    w1c = nc.declare_dram_parameter("W1c", [128, HID], bf, isOutput=False)
    w2 = nc.declare_dram_parameter("W2", [HID, HID], bf, isOutput=False)
    w3st_d = nc.declare_dram_parameter("W3st", [128, 4, 1], bf, isOutput=False)
    b1 = nc.declare_dram_parameter("b1", [128, 1], f32, isOutput=False)
    b2 = nc.declare_dram_parameter("b2", [128, 1], f32, isOutput=False)
    # out[t, s, g, c] = delta of path tile (s, g), col c
    out = nc.declare_dram_parameter(
        "out", [t_steps, npacks, 4, NT], bf, isOutput=True)

    Relu = mybir.ActivationFunctionType.Relu
    Copy = mybir.ActivationFunctionType.Copy
    add = mybir.AluOpType.add
    amax = mybir.AluOpType.max

    NP_TOT = reps * t_steps * npacks   # total pack count in the pipeline

    with tile.TileContext(nc) as tc:
        with (
            tc.tile_pool(name="consts", bufs=1) as cpool,
            tc.tile_pool(name="f", bufs=2) as fpool,
            tc.tile_pool(name="h1r", bufs=6) as h1pool,
            tc.tile_pool(name="h2r", bufs=4) as h2pool,
            tc.tile_pool(name="dst", bufs=2) as dpool,
            tc.tile_pool(name="ph1", bufs=2, space="PSUM") as ph1pool,
            tc.tile_pool(name="ph2", bufs=2, space="PSUM") as ph2pool,
        ):
            w1sb = cpool.tile([128, HID], bf, tag="w1")
            w2sb = cpool.tile([HID, HID], bf, tag="w2")
            w3sb = cpool.tile([128, 4, 1], bf, tag="w3")
            b1sb = cpool.tile([128, 1], f32, tag="b1")
            b2sb = cpool.tile([128, 1], f32, tag="b2")
            nc.gpsimd.dma_start(w1sb[:], w1c[:])
            nc.gpsimd.dma_start(w2sb[:], w2[:])
            nc.gpsimd.dma_start(w3sb[:], w3st_d[:])
            nc.gpsimd.dma_start(b1sb[:], b1[:])
            nc.gpsimd.dma_start(b2sb[:], b2[:])

            ftile_by_gc = {}

            def load_chunk(gc):
                ci = gc % nchunks
                t0 = ci * TC
                tl = min(TC, t_steps - t0)
                ftile = fpool.tile([128, TC * xw], bf, tag="f")
                for g in range(4):
                    nc.sync.dma_start(
                        ftile[32 * g:32 * g + 8, 0:tl * xw],
                        feat[g, 0:F, t0:t0 + tl, 0:xw],
                    )
                ftile_by_gc[gc] = ftile
                if gc - 2 in ftile_by_gc:
                    del ftile_by_gc[gc - 2]
                return ftile

            fcur = None
            fnxt = None
            h1aps_by_p = {}       # global pack idx -> list of 4 h1 slices
            h2r_by_p = {}         # global pack idx -> list of 4 h2r tiles

            def stage_mm1(P, fcur):
                """K=9 fused mm1 wave + h1 pair evictions for pack P."""
                gt, s = divmod(P, npacks)
                first = gt == 0
                tt = (gt % t_steps) % TC
                kk = 8 if first else 9   # no delta row at t=0
                pairs = [ph1pool.tile([128, 2 * NT], f32, tag="ph1",
                                      name=f"ph1p{P}_{p}")
                         for p in range(2)]
                PW = 8 if tiny_pe else NT
                for g in range(4):
                    fs = tt * xw + NT * s
                    nc.tensor.matmul(
                        pairs[g // 2][:, NT * (g % 2):NT * (g % 2) + PW],
                        lhsT=w1sb[32 * g:32 * g + kk, :],
                        rhs=fcur[32 * g:32 * g + kk, fs:fs + PW],
                        start=True, stop=True,
                        tile_position=(32 * g, 0),
                    )
                h1aps = []
                EW = 2 * NT if not tiny_evict else 8
                for p in range(2):
                    h1r = h1pool.tile([128, 2 * NT], bf, tag="h1r",
                                      name=f"h1r{P}_{p}")
                    if (p + P) % 2 == 0:
                        nc.scalar.activation(h1r[:, 0:EW], pairs[p][:, 0:EW],
                                             Relu, bias=b1sb[:, 0:1])
                    else:
                        nc.vector.tensor_scalar(h1r[:, 0:EW], pairs[p][:, 0:EW],
                                                b1sb[:, 0:1], 0.0, add, amax)
                    h1aps += [(h1r, 0), (h1r, NT)]
                h1aps_by_p[P] = h1aps

            def stage_mm2(P):
                """mm2 into 2x[128,1024] psum tiles + one wide h2 eviction
                per tile (ACT and DVE in parallel)."""
                h1aps = h1aps_by_p.pop(P)
                h2rs = []
                PW = 8 if tiny_pe else NT
                EW = 8 if tiny_evict else 2 * NT
                for half in range(2):
                    ph2 = ph2pool.tile([128, 2 * NT], f32, tag="ph2",
                                       name=f"ph2_{P}_{half}")
                    for gp in range(2):
                        g = 2 * half + gp
                        h1t, off = h1aps[g]
                        nc.tensor.matmul(
                            ph2[:, NT * gp:NT * gp + PW], lhsT=w2sb[:],
                            rhs=h1t[:, off:off + PW],
                            start=True, stop=True)
                    h2r = h2pool.tile([128, 2 * NT], bf, tag="h2r",
                                      name=f"h2r_{P}_{half}")
                    if (half + P) % 2 == 0:
                        nc.vector.tensor_scalar(h2r[:, 0:EW], ph2[:, 0:EW],
                                                b2sb[:, 0:1], 0.0, add, amax)
                    else:
                        nc.scalar.activation(h2r[:, 0:EW], ph2[:, 0:EW], Relu,
                                             bias=b2sb[:, 0:1])
                    h2rs += [(h2r, 0), (h2r, NT)]
                h2r_by_p[P] = h2rs

            def stage_mm3(P):
                """Col-tiled mm3 wave + per-pack pd evict, scatter, out."""
                gt, s = divmod(P, npacks)
                rep, t = divmod(gt, t_steps)
                h2rs = h2r_by_p.pop(P)
                pd = ph2pool.tile([128, 2 * NT], f32, tag="ph2",
                                  name=f"pd{P}")
                PW = 8 if tiny_pe else NT
                for g in range(4):
                    h2t, off = h2rs[g]
                    nc.tensor.matmul(
                        pd[32 * g:32 * g + 1, 0:PW],
                        lhsT=w3sb[:, g, 0:1],
                        rhs=h2t[:, off:off + PW],
                        start=True, stop=True,
                        tile_position=(0, 32 * g),
                        skip_group_check=True,
                    )
                # pd eviction (adds b3), then scatter + out DMA
                ds = dpool.tile([128, NT], bf, tag="dst", name=f"ds{P}")
                EW = 8 if tiny_evict else NT
                if P % 2 == 0:
                    nc.scalar.activation(ds[:, 0:EW], pd[:, 0:EW], Copy,
                                         bias=float(b3_value))
                else:
                    nc.vector.tensor_scalar(ds[:, 0:EW], pd[:, 0:EW],
                                            float(b3_value), None, add)
                if gt + 1 < reps * t_steps:
                    rep1, t1 = divmod(gt + 1, t_steps)
                    tt1 = t1 % TC
                    gc1 = rep1 * nchunks + t1 // TC
                    ft1 = ftile_by_gc[gc1]
                    fs1 = tt1 * xw + NT * s
                    eng = nc.sync if s % 2 == 0 else nc.gpsimd
                    eng.dma_start(
                        ft1[8:105:32, fs1:fs1 + NT],
                        ds[0:97:32, 0:NT],
                    )
                eng2 = nc.gpsimd if s % 2 == 0 else nc.sync
                eng2.dma_start(out[t, s, 0:4, 0:NT], ds[0:97:32, 0:NT])

            for P in range(NP_TOT + 3):
                # oldest work first: mm3(P-3), mm2(P-2), then the new wave.
                # Each eviction gets a full slot before its consumer issues.
                if 0 <= P - 3 < NP_TOT:
                    stage_mm3(P - 3)
                if 0 <= P - 2 < NP_TOT:
                    stage_mm2(P - 2)
                if P < NP_TOT:
                    gt, s = divmod(P, npacks)
                    rep, t = divmod(gt, t_steps)
                    tt = t % TC
                    ci = t // TC
                    if s == 0:
                        if P == 0:
                            fcur = load_chunk(0)
                        elif tt == 0:
                            fcur = fnxt
                        if tt == 0:
                            gc = rep * nchunks + ci
                            if (ci + 1) * TC < t_steps or rep + 1 < reps:
                                fnxt = load_chunk(gc + 1)
                    stage_mm1(P, fcur)

    nc.compile()
    return nc


_NC_CACHE = {}


def _get_nc(nsh=NSH, t_steps=T_FULL, num_cores=N_CORES, b3_value=0.0, **kw):
    key = (nsh, t_steps, num_cores, float(b3_value), tuple(sorted(kw.items())))
    if key not in _NC_CACHE:
        _NC_CACHE[key] = build_kernel(nsh, t_steps, num_cores, b3_value, **kw)
    return _NC_CACHE[key]


def prep_core_inputs(features, W1, b1, W2, b2, W3, b3, num_cores=N_CORES):
    """Host-side shard + repack. Returns list of per-core in_maps."""
    n, t_steps, f = features.shape
    nsh = n // num_cores
    NT = 512
    npacks = nsh // (4 * NT)
    xw = npacks * NT

    w1c = np.zeros((128, HID), dtype=BF16)
    for g in range(4):
        w1c[32 * g:32 * g + 8, :] = W1[0:8].astype(BF16)
        w1c[32 * g + 8, :] = W1[8].astype(BF16)
    w3st = W3[:, 0:1].astype(BF16).reshape(128, 1, 1).repeat(4, axis=1)
    w2b = W2.astype(BF16)
    b1c = b1.reshape(128, 1).astype(np.float32)
    b2c = b2.reshape(128, 1).astype(np.float32)

    in_maps = []
    for c in range(num_cores):
        fc = features[c * nsh:(c + 1) * nsh]          # (nsh, T, F)
        # path p = 2048s + 512g + c_ ; fpk[g, k, t, 512s + c_]
        fpk = fc.reshape(npacks, 4, NT, t_steps, f)   # (s, g, c_, t, k)
        fpk = fpk.transpose(1, 4, 3, 0, 2).reshape(4, f, t_steps, xw)
        in_maps.append({
            "features": np.ascontiguousarray(fpk).astype(BF16),
            "W1c": w1c, "W2": w2b, "W3st": w3st,
            "b1": b1c, "b2": b2c,
        })
    return in_maps


def gather_out(res_core, nsh, t_steps):
    """(T, npacks, 4, 512) bf16 -> (nsh, T) fp32, path p = 2048s+512g+c."""
    o = np.asarray(res_core).astype(np.float32)       # (t, s, g, c)
    o = o.transpose(1, 2, 3, 0)                       # (s, g, c, t)
    return o.reshape(nsh, t_steps)


def run(features, W1, b1, W2, b2, W3, b3, **run_kwargs):
    """Run on the 8 cores; returns (full_output, BassKernelResults)."""
    from concourse.bass_utils import run_bass_kernel_spmd

    features = np.asarray(features)
    n, t_steps, f = features.shape
    nsh = n // N_CORES
    b3v = float(np.asarray(b3).reshape(-1)[0])
    in_maps = prep_core_inputs(features, W1, b1, W2, b2, W3, b3)
    nc = _get_nc(nsh, t_steps, N_CORES, b3v)
    res = run_bass_kernel_spmd(nc, in_maps, core_ids=list(range(N_CORES)), **run_kwargs)
    outs = [gather_out(res.results[c]["out"], nsh, t_steps)
            for c in range(N_CORES)]
    return np.concatenate(outs, axis=0), res


def kernel(features, W1, b1, W2, b2, W3, b3):
    out, _ = run(features, W1, b1, W2, b2, W3, b3)
    return out
